# revision 4
# baseline (speedup 1.0000x reference)
"""Bass/Tile kernel for nn_Attention_50070728737153 on 8 Trainium2 cores.

Reference computation (L=2, B=32, S=2048, H=512, D=2H=1024):
    h      = hidden[-1]                              # [B, D]
    e_h    = h @ w_h.T + attn_b                      # [B, H]
    e_e    = enc @ w_e.T                             # [B, S, H]
    energy = tanh(e_e + e_h[:, None, :])             # [B, S, H]
    scores = energy @ v                              # [B, S]
    aw     = softmax(scores, axis=1)                 # [B, S]
    ctx    = einsum('bs,bsd->bd', aw, enc)           # [B, D]
    return ctx[:, None, :], aw[:, None, :]

Sharding: data-parallel over batch. 32 batches / 8 cores = 4 per core; the
small attn/v weights are replicated. No collectives needed.

Per-core dataflow (all big-matmul inputs bf16, fp32 PSUM accumulation):
  - e_e computed transposed ([H on partitions, S free]) with the w_e.T tiles
    as the stationary operand, so tanh's additive term (e_h + attn_b, which
    varies along H) rides the scalar-engine per-partition bias port.
  - scores via lhsT = v column chunks ([128, 1]) against the energy tiles.
  - softmax on a single-partition [1, S] row; exp's accum_out gives the
    denominator for free; context is computed with unnormalized exp weights
    and scaled by 1/sum at the end.
  - context via lhsT = transposed weight chunks ([128, 1]) against the
    original-layout enc tiles ([S on partitions, D free]).
"""

import os

import numpy as np
import ml_dtypes

L, B, S, H = 2, 32, 2048, 512
D = 2 * H
NCORES = 8
BL = B // NCORES  # batches per core
P = 128
DC = D // P   # 8 contraction chunks
HC = H // P   # 4 h chunks
ST = 512      # s tile (matmul free dim / PSUM bank)
NST = S // ST
NSC = S // P  # s chunks for the context matmuls

_cache = {}

LAST_EXEC_TIME_NS = None


def _build(reps=1):
    import concourse.mybir as mybir
    import concourse.tile as tile
    from concourse import bacc
    from contextlib import ExitStack

    fp32 = mybir.dt.float32
    bf16 = mybir.dt.bfloat16
    AF = mybir.ActivationFunctionType

    nc = bacc.Bacc(
        "TRN2",
        target_bir_lowering=False,
        debug=False,
        enable_asserts=False,
        num_devices=NCORES,
    )

    enc_t_ap = nc.dram_tensor("enc_t", [BL, D, S], bf16, kind="ExternalInput").ap()
    enc_o_ap = nc.dram_tensor("enc_o", [BL, S, D], bf16, kind="ExternalInput").ap()
    h_t_ap = nc.dram_tensor("h_t", [D, BL], fp32, kind="ExternalInput").ap()
    w_ht_ap = nc.dram_tensor("w_ht", [D, H], fp32, kind="ExternalInput").ap()
    w_et_ap = nc.dram_tensor("w_et", [D, H], bf16, kind="ExternalInput").ap()
    attnb_ap = nc.dram_tensor("attn_b", [H], fp32, kind="ExternalInput").ap()
    v_ap = nc.dram_tensor("v", [H], bf16, kind="ExternalInput").ap()
    out_ctx_ap = nc.dram_tensor("out_ctx", [BL, D], fp32, kind="ExternalOutput").ap()
    out_aw_ap = nc.dram_tensor("out_aw", [BL, S], fp32, kind="ExternalOutput").ap()

    with tile.TileContext(nc) as tc, ExitStack() as ctx:
        consts = ctx.enter_context(tc.tile_pool(name="consts", bufs=1))
        dram = ctx.enter_context(tc.tile_pool(name="dram", bufs=1, space="DRAM"))
        enc_t_pool = ctx.enter_context(tc.tile_pool(name="enc_t", bufs=2))
        enc_o_pool = ctx.enter_context(tc.tile_pool(name="enc_o", bufs=2))
        epool = ctx.enter_context(tc.tile_pool(name="energy", bufs=6))
        rows = ctx.enter_context(tc.tile_pool(name="rows", bufs=2))
        psum_e_pool = ctx.enter_context(tc.tile_pool(name="psum_e", bufs=4, space="PSUM"))
        psum_s_pool = ctx.enter_context(tc.tile_pool(name="psum_s", bufs=1, space="PSUM"))
        psum_c_pool = ctx.enter_context(tc.tile_pool(name="psum_c", bufs=1, space="PSUM"))
        psum_eh_pool = ctx.enter_context(tc.tile_pool(name="psum_eh", bufs=1, space="PSUM"))

        # ---- replicated constants -> SBUF
        w_et_sb = consts.tile([P, DC, H], bf16, name="w_et_sb")
        nc.sync.dma_start(w_et_sb[:], w_et_ap.rearrange("(do p) h -> p do h", p=P))
        w_ht_sb = consts.tile([P, DC, H], fp32, name="w_ht_sb")
        nc.sync.dma_start(w_ht_sb[:], w_ht_ap.rearrange("(do p) h -> p do h", p=P))
        h_t_sb = consts.tile([P, DC, BL], fp32, name="h_t_sb")
        nc.sync.dma_start(h_t_sb[:], h_t_ap.rearrange("(do p) b -> p do b", p=P))
        v_sb = consts.tile([P, HC], bf16, name="v_sb")
        nc.sync.dma_start(v_sb[:], v_ap.rearrange("(hc hi) -> hi hc", hi=P))
        attnb_sb = consts.tile([1, H], fp32, name="attnb_sb")
        nc.sync.dma_start(attnb_sb[:], attnb_ap[None, :])
        ones_sb = consts.tile([1, BL], fp32, name="ones_sb")
        nc.vector.memset(ones_sb[:], 1.0)

        # ---- e_h = h @ w_h.T + attn_b, then transpose to [h-partition, b]
        psum_eh = psum_eh_pool.tile([BL, H], fp32, name="psum_eh")
        for dc in range(DC):
            nc.tensor.matmul(
                psum_eh[:], lhsT=h_t_sb[:, dc], rhs=w_ht_sb[:, dc],
                start=(dc == 0), stop=False,
            )
        nc.tensor.matmul(
            psum_eh[:], lhsT=ones_sb[:], rhs=attnb_sb[:], start=False, stop=True
        )
        ehb_sb = consts.tile([BL, H], fp32, name="ehb_sb")
        nc.vector.tensor_copy(ehb_sb[:], psum_eh[:])
        ehb_dram = dram.tile([H, BL], fp32, name="ehb_dram")
        nc.sync.dma_start(ehb_dram.rearrange("h b -> b h"), ehb_sb[:])
        bias_all = consts.tile([P, HC, BL], fp32, name="bias_all")
        nc.sync.dma_start(
            bias_all[:], ehb_dram.rearrange("(hc hi) b -> hi hc b", hi=P)
        )

        # ---- per-batch main pipeline (reps > 1 only for benchmarking)
        for _rep in range(reps):
            for b in range(BL):
                enc_t_sb = enc_t_pool.tile([P, DC, S], bf16, name="enc_t_sb", tag="enc_t")
                nc.sync.dma_start(
                    enc_t_sb[:], enc_t_ap[b].rearrange("(do p) s -> p do s", p=P)
                )
                scores_row = rows.tile([1, S], fp32, name="scores_row", tag="scores")
                for st in range(NST):
                    psum_s = psum_s_pool.tile([1, ST], fp32, name="psum_s", tag="s")
                    for hc in range(HC):
                        psum_e = psum_e_pool.tile([P, ST], fp32, name="psum_e", tag="e")
                        for dc in range(DC):
                            nc.tensor.matmul(
                                psum_e[:],
                                lhsT=w_et_sb[:, dc, hc * P:(hc + 1) * P],
                                rhs=enc_t_sb[:, dc, st * ST:(st + 1) * ST],
                                start=(dc == 0), stop=(dc == DC - 1),
                            )
                        energyT = epool.tile([P, ST], bf16, name="energyT", tag="en")
                        nc.scalar.activation(
                            energyT[:], psum_e[:], AF.Tanh, bias=bias_all[:, hc, b:b + 1]
                        )
                        nc.tensor.matmul(
                            psum_s[:], lhsT=v_sb[:, hc:hc + 1], rhs=energyT[:],
                            start=(hc == 0), stop=(hc == HC - 1),
                        )
                    nc.vector.tensor_copy(scores_row[:, st * ST:(st + 1) * ST], psum_s[:])

                # softmax pieces on the [1, S] row
                nmax = rows.tile([1, 1], fp32, name="nmax", tag="nmax")
                nc.vector.reduce_max(
                    nmax[:], scores_row[:], axis=mybir.AxisListType.X, negate=True
                )
                p_row = rows.tile([1, S], fp32, name="p_row", tag="p")
                sumexp = rows.tile([1, 1], fp32, name="sumexp", tag="sumexp")
                nc.scalar.activation(
                    p_row[:], scores_row[:], AF.Exp, bias=nmax[:], accum_out=sumexp[:]
                )
                rinv = rows.tile([1, 1], fp32, name="rinv", tag="rinv")
                nc.vector.reciprocal(rinv[:], sumexp[:])
                aw_row = rows.tile([1, S], fp32, name="aw_row", tag="aw", bufs=1)
                nc.vector.tensor_scalar_mul(aw_row[:], p_row[:], rinv[:])
                nc.sync.dma_start(out_aw_ap[b:b + 1], aw_row[:])

                # transpose p to [128, NSC] via a DRAM bounce (fp32 lacks DMA transpose)
                p_bf = rows.tile([1, S], bf16, name="p_bf", tag="pbf", bufs=1)
                nc.vector.tensor_copy(p_bf[:], p_row[:])
                pt_dram = dram.tile([S], bf16, name="pt_dram", tag="pt", bufs=2)
                nc.sync.dma_start(pt_dram[:], p_bf[:])
                p_t = rows.tile([P, NSC], bf16, name="p_t", tag="ptsb")
                nc.sync.dma_start(p_t[:], pt_dram.rearrange("(sc si) -> si sc", si=P))

                # context = sum_s p[s] * enc[s, :]  (scaled by rinv at the end)
                psum_c0 = psum_c_pool.tile([1, ST], fp32, name="psum_c0", tag="c0")
                psum_c1 = psum_c_pool.tile([1, ST], fp32, name="psum_c1", tag="c1")
                SCH = NSC // 2  # s chunks per enc_o DMA (2 MB each)
                for half in range(2):
                    enc_o_sb = enc_o_pool.tile([P, SCH, D], bf16, name="enc_o_sb", tag="enc_o")
                    nc.sync.dma_start(
                        enc_o_sb[:],
                        enc_o_ap[b, half * SCH * P:(half + 1) * SCH * P, :].rearrange(
                            "(so p) d -> p so d", p=P
                        ),
                    )
                    for i in range(SCH):
                        sc = half * SCH + i
                        nc.tensor.matmul(
                            psum_c0[:], lhsT=p_t[:, sc:sc + 1], rhs=enc_o_sb[:, i, 0:ST],
                            start=(sc == 0), stop=(sc == NSC - 1),
                        )
                        nc.tensor.matmul(
                            psum_c1[:], lhsT=p_t[:, sc:sc + 1], rhs=enc_o_sb[:, i, ST:D],
                            start=(sc == 0), stop=(sc == NSC - 1),
                        )
                ctx_row = rows.tile([1, D], fp32, name="ctx_row", tag="ctx")
                nc.vector.tensor_scalar_mul(ctx_row[:, 0:ST], psum_c0[:], rinv[:])
                nc.vector.tensor_scalar_mul(ctx_row[:, ST:D], psum_c1[:], rinv[:])
                nc.sync.dma_start(out_ctx_ap[b:b + 1], ctx_row[:])

    nc.compile()
    return nc


def _shard_inputs(hidden, encoder_outputs, attn_w, attn_b, v_w):
    bf = ml_dtypes.bfloat16
    hidden = np.asarray(hidden, dtype=np.float32)
    enc = np.asarray(encoder_outputs, dtype=np.float32)
    attn_w = np.asarray(attn_w, dtype=np.float32)
    attn_b_np = np.asarray(attn_b, dtype=np.float32)
    v_w = np.asarray(v_w, dtype=np.float32)

    h_t = np.ascontiguousarray(hidden[-1].T)                    # [D, B] fp32
    w_ht = np.ascontiguousarray(attn_w[:, :D].T)                # [D, H] fp32
    w_et = np.ascontiguousarray(attn_w[:, D:].T.astype(bf))     # [D, H] bf16
    v = np.ascontiguousarray(v_w[0].astype(bf))                 # [H] bf16
    enc_bf = enc.astype(bf)                                     # [B, S, D]

    in_maps = []
    for c in range(NCORES):
        sl = slice(c * BL, (c + 1) * BL)
        in_maps.append({
            "enc_t": np.ascontiguousarray(enc_bf[sl].transpose(0, 2, 1)),
            "enc_o": np.ascontiguousarray(enc_bf[sl]),
            "h_t": np.ascontiguousarray(h_t[:, sl]),
            "w_ht": w_ht,
            "w_et": w_et,
            "attn_b": attn_b_np,
            "v": v,
        })
    return in_maps


def _make_runner(nc, in_maps):
    """Reusable jitted SPMD runner (mirrors bass2jax.run_bass_via_pjrt, but
    keeps inputs device-resident and the jit cache warm across calls)."""
    import jax
    import concourse.mybir as mybir
    from concourse.bass2jax import (
        _bass_exec_p,
        install_neuronx_cc_hook,
        partition_id_tensor,
    )
    from jax.experimental.shard_map import shard_map
    from jax.sharding import Mesh, NamedSharding, PartitionSpec

    install_neuronx_cc_hook()
    n_cores = len(in_maps)
    partition_name = nc.partition_id_tensor.name if nc.partition_id_tensor else None

    in_names, out_names, out_avals, zero_outs = [], [], [], []
    for alloc in nc.m.functions[0].allocations:
        if not isinstance(alloc, mybir.MemoryLocationSet):
            continue
        name = alloc.memorylocations[0].name
        if alloc.kind == "ExternalInput":
            if name != partition_name:
                in_names.append(name)
        elif alloc.kind == "ExternalOutput":
            shape = tuple(alloc.tensor_shape)
            dtype = mybir.dt.np(alloc.dtype)
            out_names.append(name)
            out_avals.append(jax.core.ShapedArray(shape, dtype))
            zero_outs.append(np.zeros(shape, dtype))
    n_params = len(in_names)
    n_outs = len(out_avals)
    all_names = list(in_names) + list(out_names)
    if partition_name is not None:
        all_names.append(partition_name)

    def _body(*args):
        operands = list(args)
        if partition_name is not None:
            operands.append(partition_id_tensor())
        outs = _bass_exec_p.bind(
            *operands,
            out_avals=tuple(out_avals),
            in_names=tuple(all_names),
            out_names=tuple(out_names),
            lowering_input_output_aliases=(),
            sim_require_finite=True,
            sim_require_nnan=True,
            nc=nc,
        )
        return tuple(outs)

    devices = jax.devices()[:n_cores]
    mesh = Mesh(np.asarray(devices), ("core",))
    in_specs = (PartitionSpec("core"),) * (n_params + n_outs)
    out_specs = (PartitionSpec("core"),) * n_outs
    donate = tuple(range(n_params, n_params + n_outs))
    sharded = jax.jit(
        shard_map(_body, mesh=mesh, in_specs=in_specs, out_specs=out_specs,
                  check_rep=False),
        donate_argnums=donate,
        keep_unused=True,
    )

    concat_in = [
        np.concatenate([np.asarray(in_maps[c][nm]) for c in range(n_cores)], axis=0)
        for nm in in_names
    ]
    sharding = NamedSharding(mesh, PartitionSpec("core"))
    dev_in = [jax.device_put(x, sharding) for x in concat_in]

    def run():
        zeros = [
            np.zeros((n_cores * z.shape[0], *z.shape[1:]), z.dtype) for z in zero_outs
        ]
        outs = sharded(*dev_in, *zeros)
        outs = [np.asarray(o) for o in jax.block_until_ready(outs)]
        return [
            {
                nm: outs[i].reshape(n_cores, *out_avals[i].shape)[c]
                for i, nm in enumerate(out_names)
            }
            for c in range(n_cores)
        ]

    return run


def kernel(hidden, encoder_outputs, attn_w, attn_b, v_w):
    in_maps = _shard_inputs(hidden, encoder_outputs, attn_w, attn_b, v_w)
    if "nc1" not in _cache:
        _cache["nc1"] = _build(reps=1)
    runner = _make_runner(_cache["nc1"], in_maps)
    results = runner()
    ctx = np.concatenate([results[c]["out_ctx"] for c in range(NCORES)], axis=0)
    aw = np.concatenate([results[c]["out_aw"] for c in range(NCORES)], axis=0)
    return ctx[:, None, :].astype(np.float32), aw[:, None, :].astype(np.float32)


def bench(inputs, k_lo=2, k_hi=10, n_runs=6):
    """Estimate per-iteration HW time via an on-device repeat loop: the
    difference between k_hi and k_lo repetitions cancels dispatch overhead."""
    import time

    in_maps = _shard_inputs(**inputs)
    walls = {}
    for k in (k_lo, k_hi):
        key = f"nc{k}"
        if key not in _cache:
            _cache[key] = _build(reps=k)
        run = _make_runner(_cache[key], in_maps)
        run()  # compile + warm
        run()
        best = float("inf")
        for _ in range(n_runs):
            t0 = time.perf_counter()
            run()
            best = min(best, time.perf_counter() - t0)
        walls[k] = best
        print(f"reps={k}: best wall {best * 1e3:.3f} ms", flush=True)
    t_iter_ns = (walls[k_hi] - walls[k_lo]) / (k_hi - k_lo) * 1e9
    return t_iter_ns, walls
